# revision 18
# baseline (speedup 1.0000x reference)
"""Trainium2 Bass kernel for nn_CSELoss (contrastive cosine-similarity loss).

Math (reference):
    x = rep_seq / max(||rep_seq||_row, 1e-8)        # [N, D], N=8192, D=128
    sims = x @ x.T - eye(N)*1e12                    # mask self-similarity
    logits = sims / 0.05
    loss = -mean_i( logits[i, tgt_i] - logsumexp_j logits[i, j] )
    with tgt_i = i ^ 1 (adjacent pair partner)

Distribution: each of the 8 cores gets the full rep_seq *rolled* by
-1024*core rows, so every core runs the identical static program on
"local rows 0..1023 vs all 8192 columns".  The diagonal / target block
always sits in local columns [t*128, (t+1)*128) for row-tile t.  Each
core returns per-row LSE and target-cosine values; the host averages
them (order-independent) to the scalar loss.

No max-subtraction is needed: logits are in [-20, 20] (plus the masked
diagonal at -2e13, whose exp is exactly 0), so row sums of exp stay
within fp32 range.
"""

import numpy as np

import concourse.bacc as bacc
import concourse.bass as bass  # noqa: F401  (AP helpers)
import concourse.tile as tile
from concourse import mybir
from concourse.bass_utils import run_bass_kernel_spmd

N, D = 8192, 128
NCORES = 8
ROWS = N // NCORES            # 1024 local rows per core
NT = ROWS // 128              # 8 row-tiles of 128 rows
NG = N // 128                 # 64 groups of 128 rows (normalization)
SG = 8                        # groups per preamble slab
NSLABS = NG // SG             # 8 slabs of 1024 rows
CHUNK = 2048                  # PSUM region width (4 banks)
NQ = N // CHUNK               # 4 column chunks
MM = 512                      # moving free dim per matmul
ALPHA = 0.05
INV_ALPHA = 1.0 / ALPHA
BIG = 1e12
EPS = 1e-8

F32 = mybir.dt.float32
BF16 = mybir.dt.bfloat16
ALU = mybir.AluOpType
AF = mybir.ActivationFunctionType


def _patch_act_tables():
    """Force every activation onto the one set that has Exp+Ln+Square, so
    the kernel pays a single ACT_TABLE_LOAD.  The greedy per-activation set
    choice otherwise ping-pongs between exp_and_others and natural_log
    (~1.3us per reload, on the bottleneck engine).  Entry positions (the
    act_func_set_id) are preserved; other sets are just emptied."""
    import concourse.bacc as _bacc

    if getattr(_bacc.get_activation_tables, "_cse_patched", False):
        return
    orig = _bacc.get_activation_tables

    def patched(module_arch):
        tabs = dict(orig(module_arch))
        keep = "natural_log_exp_and_others"
        if keep in tabs:
            need = tabs[keep]
            for name in list(tabs):
                if name != keep and (tabs[name] & need):
                    tabs[name] = tabs[name] - need
        return tabs

    patched._cse_patched = True
    _bacc.get_activation_tables = patched


SLAB_SIZES = [4, 4] + [8] * 7          # groups (of 128 rows) per slab
SLAB_STARTS = [sum(SLAB_SIZES[:i]) for i in range(len(SLAB_SIZES))]
# emit main chunk q after this many preamble slabs
MAIN_AFTER = {2: 0, 4: 1, 6: 2, 8: 3}
# PE-warmup dummy matmul regions emitted after each slab (keeps the HAM
# activity monitor at the 2.4 GHz clock before the real matmuls arrive)
WARM_AFTER = {0: 3, 1: 3, 2: 2, 3: 1, 4: 1}


def _emit(ctx, tc, rep, dneg, perm, lse_out, ctgt_out):
    nc = tc.nc

    singles = ctx.enter_context(tc.tile_pool(name="singles", bufs=1))
    rpool = ctx.enter_context(tc.tile_pool(name="rslab", bufs=3))
    sqpool = ctx.enter_context(tc.tile_pool(name="sqslab", bufs=2))
    nrmpool = ctx.enter_context(tc.tile_pool(name="nrm", bufs=6))
    xnpool = ctx.enter_context(tc.tile_pool(name="xnslab", bufs=3))
    xtpool = ctx.enter_context(tc.tile_pool(name="xt", bufs=len(SLAB_SIZES)))
    dpool = ctx.enter_context(tc.tile_pool(name="dstage",
                                           bufs=len(SLAB_SIZES), space="DRAM"))
    psum = ctx.enter_context(tc.tile_pool(name="psum", bufs=2, space="PSUM"))
    work = ctx.enter_context(tc.tile_pool(name="work", bufs=2))

    dneg_sb = singles.tile([128, 128], F32)
    nc.sync.dma_start(dneg_sb, dneg)
    perm_sb = singles.tile([128, 128], F32)
    nc.sync.dma_start(perm_sb, perm)

    sqd2 = singles.tile([128, 128], F32)    # target-extract scratch
    sums = singles.tile([128, NT * NQ], F32)
    ctgt = singles.tile([128, NT], F32)

    warm_src = singles.tile([128, MM], BF16)
    nc.vector.memset(warm_src, 0.0)

    xts = []

    def col_ap(col0, width):
        """xT AP covering [col0, col0+width) — must sit inside one slab."""
        s = 0
        while SLAB_STARTS[s] * 128 + SLAB_SIZES[s] * 128 <= col0:
            s += 1
        off = col0 - SLAB_STARTS[s] * 128
        assert off + width <= SLAB_SIZES[s] * 128
        return xts[s][:, off:off + width]

    def emit_warm(nregions):
        for _ in range(nregions):
            w = psum.tile([128, CHUNK], F32, tag="ps")
            for n in range(CHUNK // MM):
                nc.tensor.matmul(w[:, n * MM:(n + 1) * MM],
                                 warm_src[:, 0:128], warm_src,
                                 start=True, stop=True)

    def preamble_slab(s):
        sg = SLAB_SIZES[s]
        r0 = SLAB_STARTS[s] * 128
        # partition p holds the sg consecutive rows r0 + p*sg .. +sg --
        # one contiguous (sg*512)B read per partition.
        src = rep[r0:r0 + sg * 128, :].rearrange("(p n) d -> p n d", n=sg)
        rs = rpool.tile([128, sg, 128], F32, tag="rs")
        nc.scalar.dma_start(rs, src)      # ACT-side HWDGE queues: keeps the
        # loads out of the store/transpose FIFO on the sync engine
        sq = sqpool.tile([128, sg, 128], F32, tag="sq")
        nc.vector.tensor_mul(sq, rs, rs)
        ssq = nrmpool.tile([128, sg], F32, tag="nrm")
        nc.vector.tensor_reduce(ssq, sq, axis=mybir.AxisListType.X, op=ALU.add)
        # 1/norm = exp(-0.5*ln(ssq)); ln(0) -> -inf -> exp -> +inf -> min 1/eps
        lninv = nrmpool.tile([128, sg], F32, tag="nrm")
        nc.scalar.activation(lninv, ssq, AF.Ln)
        invn = nrmpool.tile([128, sg], F32, tag="nrm")
        nc.scalar.activation(invn, lninv, AF.Exp, scale=-0.5)
        nc.vector.tensor_scalar_min(invn, invn, 1.0 / EPS)
        # normalize + cast to bf16 (row layout)
        xs = xnpool.tile([128, sg, 128], BF16, tag="xs")
        for g in range(sg):
            nc.vector.tensor_scalar_mul(xs[:, g, :], rs[:, g, :],
                                        invn[:, g:g + 1])
        # bounce through DRAM to transpose into [D, rows] layout
        ds = dpool.tile([sg * 128, D], BF16, tag="ds")
        nc.sync.dma_start(ds.rearrange("(p n) d -> p n d", n=sg), xs)
        xt = xtpool.tile([128, sg * 128], BF16, tag="xt")
        nc.sync.dma_start_transpose(xt, ds)
        xts.append(xt)

    def main_chunk(q):
        for t in range(NT):
            ps = psum.tile([128, CHUNK], F32, tag="ps")
            lhsT = col_ap(t * 128, 128)
            for n in range(CHUNK // MM):
                nc.tensor.matmul(ps[:, n * MM:(n + 1) * MM], lhsT,
                                 col_ap(q * CHUNK + n * MM, MM),
                                 start=True, stop=True)
            if q == 0:
                # diagonal/target block: local cols [t*128, (t+1)*128)
                sl = ps[:, t * 128:(t + 1) * 128]
                nc.vector.tensor_mul(sqd2, sl, perm_sb)
                nc.vector.tensor_reduce(ctgt[:, t:t + 1], sqd2,
                                        axis=mybir.AxisListType.X, op=ALU.add)
                nc.vector.tensor_add(sl, sl, dneg_sb)
            ed = work.tile([128, CHUNK], BF16)
            k = t * NQ + q
            nc.scalar.activation(ed, ps, AF.Exp, scale=INV_ALPHA,
                                 accum_out=sums[:, k:k + 1])

    # interleave emission so each chunk's work follows only the slabs it
    # needs, with PE warm-up matmuls filling the preamble window
    for s in range(len(SLAB_SIZES)):
        preamble_slab(s)
        if s in WARM_AFTER:
            emit_warm(WARM_AFTER[s])
        if s in MAIN_AFTER:
            main_chunk(MAIN_AFTER[s])

    rowsum = singles.tile([128, NT], F32)
    nc.vector.tensor_reduce(rowsum, sums[:].rearrange("p (t q) -> p t q", q=NQ),
                            axis=mybir.AxisListType.X, op=ALU.add)
    lse = singles.tile([128, NT], F32)
    nc.scalar.activation(lse, rowsum, AF.Ln)
    nc.sync.dma_start(lse_out, lse)
    nc.sync.dma_start(ctgt_out, ctgt)


def build():
    _patch_act_tables()
    nc = bacc.Bacc("TRN2", target_bir_lowering=False, debug=False,
                   enable_asserts=True, num_devices=NCORES)
    rep = nc.dram_tensor("rep", [N, D], F32, kind="ExternalInput").ap()
    dneg = nc.dram_tensor("dneg", [128, 128], F32, kind="ExternalInput").ap()
    perm = nc.dram_tensor("perm", [128, 128], F32, kind="ExternalInput").ap()
    lse_out = nc.dram_tensor("lse_out", [128, NT], F32,
                             kind="ExternalOutput").ap()
    ctgt_out = nc.dram_tensor("ctgt_out", [128, NT], F32,
                              kind="ExternalOutput").ap()
    from contextlib import ExitStack
    with tile.TileContext(nc) as tc, ExitStack() as ctx:
        _emit(ctx, tc, rep, dneg, perm, lse_out, ctgt_out)
    nc.compile()
    return nc


_CACHE = {}


def get_nc():
    if "nc" not in _CACHE:
        _CACHE["nc"] = build()
    return _CACHE["nc"]


def make_in_maps(rep_seq):
    rep = np.ascontiguousarray(np.asarray(rep_seq, dtype=np.float32))
    assert rep.shape == (N, D)
    dneg = np.eye(128, dtype=np.float32) * np.float32(-BIG)
    perm = np.zeros((128, 128), dtype=np.float32)
    perm[np.arange(128), np.arange(128) ^ 1] = 1.0
    return [
        {"rep": np.roll(rep, -ROWS * c, axis=0), "dneg": dneg, "perm": perm}
        for c in range(NCORES)
    ]


def gather(results):
    lse = np.stack([r["lse_out"] for r in results]).astype(np.float64)
    ct = np.stack([r["ctgt_out"] for r in results]).astype(np.float64)
    loss = lse.mean() - INV_ALPHA * ct.mean()
    return np.asarray(loss, dtype=np.float32)


def _ensure_ntff_hook():
    """The agent image's antenv lacks axon_hooks; synthesize it so
    run_bass_kernel_spmd(trace=True) can NTFF-profile via libaxon."""
    import importlib
    import os
    import sys
    import types

    try:
        importlib.import_module("antenv.axon_hooks")
        return
    except ImportError:
        pass
    try:
        from trn_agent_boot.trn_boot import _ntff_profile_via_ctypes
    except ImportError:
        return
    so_path = os.environ.get("PJRT_LIBRARY_PATH", "/opt/axon/libaxon_pjrt.so")
    if not os.path.exists(so_path):
        return
    hook = _ntff_profile_via_ctypes(so_path)
    mod = types.ModuleType("antenv.axon_hooks")
    mod._hook = hook
    mod.get_axon_ntff_profile_hook = lambda: mod._hook
    mod.set_axon_ntff_profile_hook = lambda h: setattr(mod, "_hook", h)
    sys.modules["antenv.axon_hooks"] = mod


def run(rep_seq, trace=False):
    nc = get_nc()
    if trace:
        _ensure_ntff_hook()
    res = run_bass_kernel_spmd(nc, make_in_maps(rep_seq),
                               core_ids=list(range(NCORES)), trace=trace)
    return gather(res.results), res


def kernel(**inputs):
    loss, _ = run(inputs["rep_seq"])
    return loss


# revision 20
# speedup vs baseline: 1.2438x; 1.2438x over previous
"""Trainium2 Bass kernel for nn_CSELoss (contrastive cosine-similarity loss).

Math (reference):
    x = rep_seq / max(||rep_seq||_row, 1e-8)        # [N, D], N=8192, D=128
    sims = x @ x.T - eye(N)*1e12                    # mask self-similarity
    logits = sims / 0.05
    loss = -mean_i( logits[i, tgt_i] - logsumexp_j logits[i, j] )
    with tgt_i = i ^ 1 (adjacent pair partner)

Distribution: each of the 8 cores gets the full rep_seq *rolled* by
-1024*core rows, so every core runs the identical static program on
"local rows 0..1023 vs all 8192 columns".  The diagonal block always
sits in local columns [t*128, (t+1)*128) for row-tile t.  Each core
returns per-row sums of exp plus per-pair target cosines; the host
averages them (order-independent) into the scalar loss.

No max-subtraction is needed: logits are in [-20, 20] (plus the masked
diagonal at about -2e13, whose exp is exactly 0), so row sums of exp
stay within fp32 range.

Structure per core:
  preamble (9 slabs of 512/1024 rows):
    DMA rows -> DVE sum-of-squares -> ACT exp(-0.5*ln(ssq)) = 1/norm ->
    DVE scale+cast to bf16 -> DMA to DRAM -> DMA-xbar-transpose back as
    xT[:, slab] (the [D, N] operand); plus DVE pair-dot for the target
    cosines straight from the row layout.
  main loop (4 column chunks x 8 row tiles):
    4 matmuls fill a [128, 2048] PSUM region; the diagonal 128-block
    gets an extra accumulating matmul (2^20 I).T @ (-2^20 I) = -2^40 I
    (PE-side masking -- no vector op touches the main loop); ACT does
    exp(20x) with a fused per-row accumulator.
  PE warm-up matmuls run during the preamble so HAM is at 2.4 GHz when
  the real matmuls arrive.
"""

import numpy as np

import concourse.bacc as bacc
import concourse.bass as bass  # noqa: F401
import concourse.tile as tile
from concourse import mybir
from concourse.bass_utils import run_bass_kernel_spmd

N, D = 8192, 128
NCORES = 8
ROWS = N // NCORES            # 1024 local rows per core
NT = ROWS // 128              # 8 row-tiles of 128 rows
CHUNK = 2048                  # PSUM region width (4 banks)
NQ = N // CHUNK               # 4 column chunks
MM = 512                      # moving free dim per matmul
ALPHA = 0.05
INV_ALPHA = 1.0 / ALPHA
EPS = 1e-8
CDIAG = float(2 ** 20)        # (2^20)^2 = 2^40 ~ 1.1e12 diagonal mask

F32 = mybir.dt.float32
BF16 = mybir.dt.bfloat16
ALU = mybir.AluOpType
AF = mybir.ActivationFunctionType

SLAB_SIZES = [4, 4] + [8] * 7          # groups (of 128 rows) per slab
SLAB_STARTS = [sum(SLAB_SIZES[:i]) for i in range(len(SLAB_SIZES))]
NPAIR = N // 256                       # pair-columns of ctgt output (32)


def _patch_act_tables():
    """Force every activation onto the one set that has Exp+Ln+Square, so
    the kernel pays a single ACT_TABLE_LOAD.  The greedy per-activation set
    choice otherwise ping-pongs between exp_and_others and natural_log
    (~1.3us per reload, on the bottleneck engine).  Entry positions (the
    act_func_set_id) are preserved; other sets are just trimmed."""
    import concourse.bacc as _bacc

    if getattr(_bacc.get_activation_tables, "_cse_patched", False):
        return
    orig = _bacc.get_activation_tables

    def patched(module_arch):
        tabs = dict(orig(module_arch))
        keep = "natural_log_exp_and_others"
        if keep in tabs:
            need = tabs[keep]
            for name in list(tabs):
                if name != keep and (tabs[name] & need):
                    tabs[name] = tabs[name] - need
        return tabs

    patched._cse_patched = True
    _bacc.get_activation_tables = patched


def _emit(ctx, tc, rep, cipos, cineg, lse_out, ctgt_out):
    nc = tc.nc

    singles = ctx.enter_context(tc.tile_pool(name="singles", bufs=1))
    rpool = ctx.enter_context(tc.tile_pool(name="rslab", bufs=3))
    sqpool = ctx.enter_context(tc.tile_pool(name="sqslab", bufs=2))
    nrmpool = ctx.enter_context(tc.tile_pool(name="nrm", bufs=6))
    xnpool = ctx.enter_context(tc.tile_pool(name="xnslab", bufs=3))
    xtpool = ctx.enter_context(tc.tile_pool(name="xt", bufs=len(SLAB_SIZES)))
    dpool = ctx.enter_context(tc.tile_pool(name="dstage",
                                           bufs=len(SLAB_SIZES), space="DRAM"))
    psum = ctx.enter_context(tc.tile_pool(name="psum", bufs=2, space="PSUM"))
    work = ctx.enter_context(tc.tile_pool(name="work", bufs=2))

    cip_sb = singles.tile([128, 128], BF16)
    nc.sync.dma_start(cip_sb, cipos)
    cin_sb = singles.tile([128, 128], BF16)
    nc.sync.dma_start(cin_sb, cineg)

    sums = singles.tile([128, NT * NQ], F32)
    ctgt = singles.tile([128, NPAIR], F32)

    warm_src = singles.tile([128, MM], BF16)
    nc.vector.memset(warm_src, 0.0)

    xts = []

    def col_ap(col0, width):
        """xT AP covering [col0, col0+width) — must sit inside one slab."""
        s = 0
        while (SLAB_STARTS[s] + SLAB_SIZES[s]) * 128 <= col0:
            s += 1
        off = col0 - SLAB_STARTS[s] * 128
        assert off + width <= SLAB_SIZES[s] * 128
        return xts[s][:, off:off + width]

    def emit_warm(nregions):
        for _ in range(nregions):
            w = psum.tile([128, CHUNK], F32, tag="ps")
            for n in range(CHUNK // MM):
                nc.tensor.matmul(w[:, n * MM:(n + 1) * MM],
                                 warm_src[:, 0:128], warm_src,
                                 start=True, stop=True)

    def preamble_slab(s):
        sg = SLAB_SIZES[s]
        r0 = SLAB_STARTS[s] * 128
        # partition p holds the sg consecutive rows r0 + p*sg .. +sg --
        # one contiguous (sg*512)B read per partition.
        src = rep[r0:r0 + sg * 128, :].rearrange("(p n) d -> p n d", n=sg)
        rs = rpool.tile([128, sg, 128], F32, tag="rs")
        nc.scalar.dma_start(rs, src)      # ACT-side HWDGE queues: keeps the
        # loads out of the store/transpose FIFO on the sync engine
        sq = sqpool.tile([128, sg, 128], F32, tag="sq")
        nc.vector.tensor_mul(sq, rs, rs)
        ssq = nrmpool.tile([128, sg], F32, tag="nrm")
        nc.vector.tensor_reduce(ssq, sq, axis=mybir.AxisListType.X, op=ALU.add)
        # 1/norm = exp(-0.5*ln(ssq)); ln(0) -> -inf -> exp -> +inf -> min 1/eps
        lninv = nrmpool.tile([128, sg], F32, tag="nrm")
        nc.scalar.activation(lninv, ssq, AF.Ln)
        invn = nrmpool.tile([128, sg], F32, tag="nrm")
        nc.scalar.activation(invn, lninv, AF.Exp, scale=-0.5)
        nc.vector.tensor_scalar_min(invn, invn, 1.0 / EPS)
        # normalize + cast to bf16 (row layout)
        xs = xnpool.tile([128, sg, 128], BF16, tag="xs")
        for g in range(sg):
            nc.vector.tensor_scalar_mul(xs[:, g, :], rs[:, g, :],
                                        invn[:, g:g + 1])
        # target cosines: rows p*sg+2h and p*sg+2h+1 are an adjacent pair
        # in the same partition; one dot per pair covers both rows.
        xsv = xs.rearrange("p (h two) d -> p h two d", two=2)
        pd = sqpool.tile([128, sg // 2, 128], F32, tag="pd")
        nc.vector.tensor_mul(pd, xsv[:, :, 0, :], xsv[:, :, 1, :])
        pb = r0 // 256
        nc.vector.tensor_reduce(ctgt[:, pb:pb + sg // 2], pd,
                                axis=mybir.AxisListType.X, op=ALU.add)
        # bounce through DRAM to transpose into [D, rows] layout
        ds = dpool.tile([sg * 128, D], BF16, tag="ds")
        nc.sync.dma_start(ds.rearrange("(p n) d -> p n d", n=sg), xs)
        xt = xtpool.tile([128, sg * 128], BF16, tag="xt")
        nc.sync.dma_start_transpose(xt, ds)
        xts.append(xt)

    def main_chunk(q):
        for t in range(NT):
            ps = psum.tile([128, CHUNK], F32, tag="ps")
            lhsT = col_ap(t * 128, 128)
            for n in range(CHUNK // MM):
                nc.tensor.matmul(ps[:, n * MM:(n + 1) * MM], lhsT,
                                 col_ap(q * CHUNK + n * MM, MM),
                                 start=True, stop=True)
            if q == 0:
                # accumulate -2^40*I onto the diagonal 128-block (PE-side
                # masking; keeps DVE entirely out of the main loop)
                off = t * 128
                nc.tensor.matmul(ps[:, off:off + 128], cip_sb, cin_sb,
                                 start=False, stop=True, skip_group_check=True)
            ed = work.tile([128, CHUNK], BF16)
            k = t * NQ + q
            nc.scalar.activation(ed, ps, AF.Exp, scale=INV_ALPHA,
                                 accum_out=sums[:, k:k + 1])

    # preamble first (PE warm-up fills the otherwise idle PE), then the
    # ACT-paced main loop; no main-loop op ever blocks a preamble stream.
    for s in range(len(SLAB_SIZES)):
        preamble_slab(s)
        if s == 0:
            emit_warm(8)
    for q in range(NQ):
        main_chunk(q)

    rowsum = singles.tile([128, NT], F32)
    nc.vector.tensor_reduce(rowsum, sums[:].rearrange("p (t q) -> p t q", q=NQ),
                            axis=mybir.AxisListType.X, op=ALU.add)
    lse = singles.tile([128, NT], F32)
    nc.scalar.activation(lse, rowsum, AF.Ln)
    nc.sync.dma_start(lse_out, lse)
    nc.sync.dma_start(ctgt_out, ctgt)


def build():
    _patch_act_tables()
    nc = bacc.Bacc("TRN2", target_bir_lowering=False, debug=False,
                   enable_asserts=True, num_devices=NCORES)
    rep = nc.dram_tensor("rep", [N, D], F32, kind="ExternalInput").ap()
    cipos = nc.dram_tensor("cipos", [128, 128], BF16,
                           kind="ExternalInput").ap()
    cineg = nc.dram_tensor("cineg", [128, 128], BF16,
                           kind="ExternalInput").ap()
    lse_out = nc.dram_tensor("lse_out", [128, NT], F32,
                             kind="ExternalOutput").ap()
    ctgt_out = nc.dram_tensor("ctgt_out", [128, NPAIR], F32,
                              kind="ExternalOutput").ap()
    from contextlib import ExitStack
    with tile.TileContext(nc) as tc, ExitStack() as ctx:
        _emit(ctx, tc, rep, cipos, cineg, lse_out, ctgt_out)
    nc.compile()
    return nc


_CACHE = {}


def get_nc():
    if "nc" not in _CACHE:
        _CACHE["nc"] = build()
    return _CACHE["nc"]


def make_in_maps(rep_seq):
    import ml_dtypes

    rep = np.ascontiguousarray(np.asarray(rep_seq, dtype=np.float32))
    assert rep.shape == (N, D)
    eye = np.eye(128, dtype=np.float32)
    cipos = (eye * CDIAG).astype(ml_dtypes.bfloat16)
    cineg = (eye * -CDIAG).astype(ml_dtypes.bfloat16)
    return [
        {"rep": np.roll(rep, -ROWS * c, axis=0), "cipos": cipos,
         "cineg": cineg}
        for c in range(NCORES)
    ]


def gather(results):
    lse = np.stack([r["lse_out"] for r in results]).astype(np.float64)
    ct = np.stack([r["ctgt_out"] for r in results]).astype(np.float64)
    loss = lse.mean() - INV_ALPHA * ct.mean()
    return np.asarray(loss, dtype=np.float32)


def _ensure_ntff_hook():
    """The agent image's antenv lacks axon_hooks; synthesize it so
    run_bass_kernel_spmd(trace=True) can NTFF-profile via libaxon."""
    import importlib
    import os
    import sys
    import types

    try:
        importlib.import_module("antenv.axon_hooks")
        return
    except ImportError:
        pass
    try:
        from trn_agent_boot.trn_boot import _ntff_profile_via_ctypes
    except ImportError:
        return
    so_path = os.environ.get("PJRT_LIBRARY_PATH", "/opt/axon/libaxon_pjrt.so")
    if not os.path.exists(so_path):
        return
    hook = _ntff_profile_via_ctypes(so_path)
    mod = types.ModuleType("antenv.axon_hooks")
    mod._hook = hook
    mod.get_axon_ntff_profile_hook = lambda: mod._hook
    mod.set_axon_ntff_profile_hook = lambda h: setattr(mod, "_hook", h)
    sys.modules["antenv.axon_hooks"] = mod


def run(rep_seq, trace=False):
    nc = get_nc()
    if trace:
        _ensure_ntff_hook()
    res = run_bass_kernel_spmd(nc, make_in_maps(rep_seq),
                               core_ids=list(range(NCORES)), trace=trace)
    return gather(res.results), res


def kernel(**inputs):
    loss, _ = run(inputs["rep_seq"])
    return loss
